# revision 1
# baseline (speedup 1.0000x reference)
"""Trainium2 Bass kernel for nn_DLCF_DCA (scatter_memory).

Reference computation, per sample b (B=128, S=256, H=768, K=64):
  keep_dep[s]  = (s==0) or any_k(depend[b,k] == s-1)
  keep_dpd[s]  = (s==0) or any_k(depended[b,k] == s-1)
  mult[s]      = w2 if s-1 in depended else (w1 if s-1 in depend else 0);
                 0 if s-1 in no_connect; 1 if s==0
  y1 = x * keep_dep;  y2 = x * keep_dpd;  y3 = x * mult

Strategy: pure data parallel over batch (16 samples per core, 8 cores).
Each core streams its [4096, 768] shard with 32 consecutive token-rows per
SBUF partition so every DMA moves long contiguous chunks per partition.
Multiplier masks are built in the matching [partition, row-in-partition]
layout: each index k is decomposed as (q, r) = divmod(b*256 + idx + 1, 32)
and membership counts come from one-hot compares contracted on the tensor
engine (count[p, r] = sum_k Q[k,p] * R[k,r]). The bulk work is then three
per-partition-scalar multiplies per 768-wide row block (vector + scalar
engines) between streamed input and output DMAs.
"""

import contextlib
import os
import sys

import numpy as np

if "/opt/trn_rl_repo" not in sys.path:
    sys.path.insert(0, "/opt/trn_rl_repo")

N_CORES = 8
B, S, H, K = 128, 256, 768, 64
BL = B // N_CORES          # samples per core
ROWS = BL * S              # 4096 token-rows per core
RPP = ROWS // 128          # 32 consecutive rows per partition
ND = 16                    # DMA tiles over the free dim
RPT = RPP // ND            # 4 row-blocks per tile
NCHUNK = BL * K // 128     # 8 contraction chunks for membership counts

_cache = {}


def _split_multiwaits(nc, max_waits=1):
    """walrus in this container only accepts one sync-wait per instruction;
    splice extra waits onto single-wait NoOps just before the offender."""
    from concourse import mybir

    n = 0
    for func in nc.m.functions:
        for bb in func.blocks:
            insts = bb.instructions
            i = 0
            while i < len(insts):
                ins = insts[i]
                si = getattr(ins, "sync_info", None)
                if si is None or len(si.on_wait) <= max_waits:
                    i += 1
                    continue
                waits = list(si.on_wait)
                keep = waits[-max_waits:]
                extra = waits[:-max_waits]
                nops = []
                for j in range(0, len(extra), max_waits):
                    n += 1
                    nops.append(
                        mybir.InstNoOp(
                            name=f"{ins.name}-ws{n}",
                            sync_info=mybir.SyncInfo(
                                on_wait=extra[j : j + max_waits], on_update=[]
                            ),
                            bass_nofuse=True,
                            engine=ins.engine,
                            ins=[],
                            outs=[],
                        )
                    )
                si.on_wait = keep
                for k, nop in enumerate(nops):
                    insts.insert(i + k, nop)
                i += len(nops) + 1
    return n


def _build():
    import concourse.bass as bass
    import concourse.tile as tile
    from concourse import mybir

    f32 = mybir.dt.float32
    i32 = mybir.dt.int32
    eq = mybir.AluOpType.is_equal
    nc = bass.Bass()

    x = nc.dram_tensor("x", [ROWS, H], f32, kind="ExternalInput")
    # meta[p, :]: q/r per list ([p, chunk]), then p0q, p0r, w1p, w2p columns
    NMETA = 6 * NCHUNK + 4
    meta = nc.dram_tensor("meta", [128 * NMETA], f32, kind="ExternalInput")
    ys = [nc.dram_tensor(f"y{i}", [ROWS, H], f32, kind="ExternalOutput")
          for i in (1, 2, 3)]

    with tile.TileContext(nc) as tc, contextlib.ExitStack() as ctx:
        const = ctx.enter_context(tc.tile_pool(name="const", bufs=1))
        epool = ctx.enter_context(tc.tile_pool(name="epool", bufs=2))
        psum = ctx.enter_context(tc.tile_pool(name="psum", bufs=1, space="PSUM"))
        xpool = ctx.enter_context(tc.tile_pool(name="xpool", bufs=ND))
        ypool = ctx.enter_context(tc.tile_pool(name="ypool", bufs=3))

        # --- small loads / iota -----------------------------------------
        mt = const.tile([128, NMETA], f32, name="meta")
        nc.sync.dma_start(out=mt[:], in_=meta.rearrange("(p c) -> p c", p=128))
        qT = {n: mt[:, i * NCHUNK : (i + 1) * NCHUNK]
              for i, n in enumerate(("dep", "dpd", "noc"))}
        rT = {n: mt[:, (3 + i) * NCHUNK : (4 + i) * NCHUNK]
              for i, n in enumerate(("dep", "dpd", "noc"))}
        p0qT = mt[:, 6 * NCHUNK : 6 * NCHUNK + 1]
        p0rT = mt[:, 6 * NCHUNK + 1 : 6 * NCHUNK + 2]
        w1T = mt[:, 6 * NCHUNK + 2 : 6 * NCHUNK + 3]
        w2T = mt[:, 6 * NCHUNK + 3 : 6 * NCHUNK + 4]

        iotai = const.tile([128, 128], i32)
        nc.gpsimd.iota(iotai[:], pattern=[[1, 128]], base=0, channel_multiplier=0)
        iota = const.tile([128, 128], f32)
        nc.vector.tensor_copy(iota[:], iotai[:])  # 0..127 along free dim
        iota2 = const.tile([128, RPP], f32)
        nc.gpsimd.tensor_copy(iota2[:], iota[:, 0:RPP])

        # --- all x tiles stream in on the ACT HWDGE ring -----------------
        NDR = ND                     # read tile granularity
        RPR = RPP // NDR             # row-blocks per read tile
        xrr = x.rearrange("(p d q) h -> d p (q h)", p=128, d=NDR)
        yr = [y.rearrange("(p d q) h -> d p (q h)", p=128, d=ND) for y in ys]
        xts = {}
        for d in range(NDR):
            xts[d] = xpool.tile([128, RPR * H], f32, name="xt")
            nc.scalar.dma_start(out=xts[d][:], in_=xrr[d])

        # --- membership counts, produced incrementally -------------------
        # order: p0, dep -> m1; dpd -> m2; noc -> m3, so the multiply/write
        # stream for y1 starts as early as possible.
        mul = mybir.AluOpType.mult
        add = mybir.AluOpType.add
        mn = mybir.AluOpType.min
        mx = mybir.AluOpType.max
        def count_list(qcol_list, rcol_list, pcname):
            pct = psum.tile([128, RPP], f32, name=pcname)  # own PSUM bank
            nchunk = len(qcol_list)
            for c in range(nchunk):
                qt = epool.tile([128, 128], f32, name="Q")
                nc.vector.tensor_scalar(qt[:], iota[:], qcol_list[c], None, op0=eq)
                rt = epool.tile([128, RPP], f32, name="R")
                nc.vector.tensor_scalar(rt[:], iota2[:], rcol_list[c], None, op0=eq)
                nc.tensor.matmul(pct[:], lhsT=qt[:], rhs=rt[:],
                                 start=(c == 0), stop=(c == nchunk - 1))
            return pct

        def cols(ap):
            return [ap[:, c : c + 1] for c in range(NCHUNK)]

        pc_p0 = count_list([p0qT], [p0rT], "pc_p0")
        p0m = const.tile([128, RPP], f32)
        nc.vector.tensor_copy(p0m[:], pc_p0[:])  # already 0/1

        pc_dep = count_list(cols(qT["dep"]), cols(rT["dep"]), "pc_dep")
        dep1 = const.tile([128, RPP], f32)
        nc.vector.tensor_scalar(dep1[:], pc_dep[:], 1.0, None, op0=mn)
        m1 = const.tile([128, RPP], f32)
        nc.vector.tensor_tensor(m1[:], dep1[:], p0m[:], op=mx)

        # multiply/write helpers (y1/y2 on vector, y3 on scalar engine)
        def xsrc(r):
            return xts[r // RPR][:, (r % RPR) * H : (r % RPR + 1) * H]

        def emit_dve(yi, mt_, d):
            yt = ypool.tile([128, RPT * H], f32, name=f"y{yi}t")
            for g in range(RPT):
                r = d * RPT + g
                blk = slice(g * H, (g + 1) * H)
                nc.vector.tensor_scalar(
                    yt[:, blk], xsrc(r), mt_[:, r : r + 1], None, op0=mul
                )
            nc.sync.dma_start(out=yr[yi - 1][d], in_=yt[:])

        def emit_act(mt_, d):
            yt = ypool.tile([128, RPT * H], f32, name="y3t")
            for g in range(RPT):
                r = d * RPT + g
                blk = slice(g * H, (g + 1) * H)
                nc.scalar.activation(
                    yt[:, blk], xsrc(r),
                    mybir.ActivationFunctionType.Copy,
                    scale=mt_[:, r : r + 1],
                )
            nc.sync.dma_start(out=yr[2][d], in_=yt[:])

        for d in range(4):
            emit_dve(1, m1, d)

        pc_dpd = count_list(cols(qT["dpd"]), cols(rT["dpd"]), "pc_dpd")
        dpd1 = const.tile([128, RPP], f32)
        nc.vector.tensor_scalar(dpd1[:], pc_dpd[:], 1.0, None, op0=mn)
        m2 = const.tile([128, RPP], f32)
        nc.vector.tensor_tensor(m2[:], dpd1[:], p0m[:], op=mx)

        for d in range(4):
            emit_dve(1, m1, d + 4)
            emit_dve(2, m2, d)

        pc_noc = count_list(cols(qT["noc"]), cols(rT["noc"]), "pc_noc")
        # m3 = ((dep1*w1)*(1-dpd1) + dpd1*w2) * (1-noc1); then 1 at s==0
        m3 = const.tile([128, RPP], f32)
        inv = const.tile([128, RPP], f32)
        tmp = const.tile([128, RPP], f32)
        nc.vector.tensor_scalar(m3[:], dep1[:], w1T, None, op0=mul)
        nc.vector.tensor_scalar(inv[:], dpd1[:], -1.0, 1.0, op0=mul, op1=add)
        nc.vector.tensor_tensor(m3[:], m3[:], inv[:], op=mul)
        nc.vector.tensor_scalar(tmp[:], dpd1[:], w2T, None, op0=mul)
        nc.vector.tensor_tensor(m3[:], m3[:], tmp[:], op=add)
        nc.vector.tensor_scalar(inv[:], pc_noc[:], 1.0, None, op0=mn)  # noc1
        nc.vector.tensor_scalar(inv[:], inv[:], -1.0, 1.0, op0=mul, op1=add)
        nc.vector.tensor_tensor(m3[:], m3[:], inv[:], op=mul)
        nc.vector.tensor_scalar(inv[:], p0m[:], -1.0, 1.0, op0=mul, op1=add)
        nc.vector.tensor_tensor(m3[:], m3[:], inv[:], op=mul)
        nc.vector.tensor_tensor(m3[:], m3[:], p0m[:], op=add)

        # --- remaining phases --------------------------------------------
        for d in range(4, ND - 4):
            emit_dve(1, m1, d + 4)
            emit_dve(2, m2, d)
        for d in range(ND - 4, ND):
            emit_dve(2, m2, d)
        for d in range(ND):
            emit_act(m3, d)

    _split_multiwaits(nc)
    return nc


def _prep_inputs(bert_local_out, depend, depended, no_connect,
                 depend_weight, depended_weight):
    x = np.ascontiguousarray(np.asarray(bert_local_out, dtype=np.float32))
    idx = {
        "dep": np.asarray(depend, dtype=np.int64),
        "dpd": np.asarray(depended, dtype=np.int64),
        "noc": np.asarray(no_connect, dtype=np.int64),
    }
    w1 = np.asarray(depend_weight, dtype=np.float32)
    w2 = np.asarray(depended_weight, dtype=np.float32)

    p0q = np.full(128, 9999.0, dtype=np.float32)
    p0r = np.full(128, 9999.0, dtype=np.float32)
    p0q[:BL] = 8 * np.arange(BL)
    p0r[:BL] = 0.0

    pidx = np.arange(128) // (128 // BL)  # sample owning each partition
    boff = np.arange(BL, dtype=np.int64)[:, None] * S  # b*256
    NMETA = 6 * NCHUNK + 4

    in_maps = []
    for c in range(N_CORES):
        sl = slice(c * BL, (c + 1) * BL)
        meta = np.empty((128, NMETA), dtype=np.float32)
        for i, n in enumerate(("dep", "dpd", "noc")):
            g = (idx[n][sl] + boff + 1).reshape(-1)  # global position + 1
            meta[:, i * NCHUNK : (i + 1) * NCHUNK] = (
                (g // RPP).astype(np.float32).reshape(NCHUNK, 128).T
            )
            meta[:, (3 + i) * NCHUNK : (4 + i) * NCHUNK] = (
                (g % RPP).astype(np.float32).reshape(NCHUNK, 128).T
            )
        meta[:, 6 * NCHUNK] = p0q
        meta[:, 6 * NCHUNK + 1] = p0r
        meta[:, 6 * NCHUNK + 2] = w1[sl][pidx]
        meta[:, 6 * NCHUNK + 3] = w2[sl][pidx]
        in_maps.append({
            "x": x[sl].reshape(ROWS, H),
            "meta": np.ascontiguousarray(meta).reshape(-1),
        })
    return in_maps


def kernel(bert_local_out, depend, depended, no_connect,
           depend_weight, depended_weight):
    from concourse.bass_utils import run_bass_kernel_spmd

    if "nc" not in _cache:
        _cache["nc"] = _build()
    nc = _cache["nc"]

    in_maps = _prep_inputs(bert_local_out, depend, depended, no_connect,
                           depend_weight, depended_weight)

    pdir = os.environ.get("KERNEL_PROFILE_DIR")
    ctx = contextlib.nullcontext()
    if pdir:
        import concourse.bass2jax as b2j
        from trn_agent_boot.trn_boot import _ntff_profile_via_ctypes

        if not getattr(b2j, "_neff_capture_patched", False):
            orig = b2j.rename_neff_tensors_and_patch_header

            def patched(neff_path, mapping):
                data = orig(neff_path, mapping)
                cap = os.environ.get("KERNEL_PROFILE_DIR")
                if cap:
                    os.makedirs(cap, exist_ok=True)
                    with open(os.path.join(cap, "model.neff"), "wb") as f:
                        f.write(data)
                return data

            b2j.rename_neff_tensors_and_patch_header = patched
            b2j._neff_capture_patched = True
        os.makedirs(pdir, exist_ok=True)
        hookf = _ntff_profile_via_ctypes("/opt/axon/libaxon_pjrt.so")
        if hookf is not None:
            dev = None if os.environ.get("KERNEL_PROFILE_ALL") else [0]
            ctx = hookf(pdir, dev)

    with ctx:
        res = run_bass_kernel_spmd(nc, in_maps, list(range(N_CORES)))

    outs = []
    for name in ("y1", "y2", "y3"):
        full = np.empty((B, S, H), dtype=np.float32)
        for c in range(N_CORES):
            full[c * BL : (c + 1) * BL] = res.results[c][name].reshape(BL, S, H)
        outs.append(full)
    return tuple(outs)



# revision 9
# speedup vs baseline: 1.3860x; 1.3860x over previous
"""Trainium2 Bass kernel for nn_DLCF_DCA (scatter_memory).

Reference, per sample b (B=128, S=256, H=768, K=64):
  keep_dep[s]  = (s==0) or any_k(depend[b,k] == s-1)
  keep_dpd[s]  = (s==0) or any_k(depended[b,k] == s-1)
  mult[s]      = 1 at s==0; 0 if s-1 in no_connect; else w2 if s-1 in
                 depended, else w1 if s-1 in depend, else 0
  y1 = x * keep_dep;  y2 = x * keep_dpd;  y3 = x * mult

All three outputs are mostly zero rows (~22-31% nonzero).  The runtime
donates pre-zeroed output buffers (see bass2jax.run_bass_via_pjrt), so the
kernel only touches the nonzero rows:

  per core (16 samples, x shard [4096, 768] in bf16):
    dma_gather the y1/y2/y3 row sets from HBM into SBUF (row j lands at
    partition j%128, slot j//128), scale the y3 rows by a per-row scalar
    table on the vector engine, then dma_scatter_add each set onto the
    zeroed outputs.  Index tables are int16 [128, N/16] (j at [j%16,
    j//16], replicated across the 8 gpsimd cores), padded with trailing
    -1 which the ucode trims per core at runtime.

Host converts x to bf16 and upcasts results to f32 (rel err ~2^-8, well
inside the 2e-2 gate); index/scale tables are assembled on the host from
the int index lists.  Pure data parallel over batch; shapes of the SPMD
program depend only on the max row counts across cores (cached on that).
"""

import contextlib
import os
import sys

import numpy as np

if "/opt/trn_rl_repo" not in sys.path:
    sys.path.insert(0, "/opt/trn_rl_repo")

import ml_dtypes

N_CORES = 8
B, S, H, K = 128, 256, 768, 64
BL = B // N_CORES          # samples per core
ROWS = BL * S              # 4096 rows per core

_cache = {}


MAXN = 1024  # dma_gather/scatter_add limit on num_idxs per instruction


def _chunk_sizes(n):
    """Split n into even 16-aligned chunks of at most MAXN."""
    k = (n + MAXN - 1) // MAXN
    per = ((n + k - 1) // k + 15) // 16 * 16
    out = []
    left = n
    for _ in range(k):
        c = min(per, max(16, (left + 15) // 16 * 16))
        out.append(c)
        left -= c
    return out


def _slots(n):
    return (n + 127) // 128


def _build(n1, n2, n3):
    """n1/n2/n3: static num_idxs per stream (multiples of 16)."""
    import concourse.bacc as bacc
    import concourse.tile as tile
    from concourse import mybir

    f32 = mybir.dt.float32
    bf16 = mybir.dt.bfloat16
    i16 = mybir.dt.int16
    mul = mybir.AluOpType.mult

    chunks = [_chunk_sizes(n) for n in (n1, n2, n3)]
    cols = [sum(c // 16 for c in ch) for ch in chunks]
    sl3 = sum(_slots(c) for c in chunks[2])

    nc = bacc.Bacc(None)
    x = nc.dram_tensor("x", [ROWS, H], bf16, kind="ExternalInput")
    idxs = [nc.dram_tensor(f"idx{i + 1}", [128, cols[i]], i16, kind="ExternalInput")
            for i in range(3)]
    scal = nc.dram_tensor("scal", [128, sl3], f32, kind="ExternalInput")
    ys = [nc.dram_tensor(f"y{i}", [ROWS, H], bf16, kind="ExternalOutput")
          for i in (1, 2, 3)]

    with tile.TileContext(nc) as tc, contextlib.ExitStack() as ctx:
        pool = ctx.enter_context(tc.tile_pool(name="pool", bufs=1))

        its = []
        for i in range(3):
            it = pool.tile([128, cols[i]], i16, name=f"i{i + 1}")
            nc.sync.dma_start(out=it[:], in_=idxs[i][:])
            its.append(it)
        sc = pool.tile([128, sl3], f32, name="sc")
        nc.sync.dma_start(out=sc[:], in_=scal[:])

        # gathers: within a chunk, row j -> partition j%128, slot j//128
        gts, gidx = [], []
        for i in range(3):
            col0 = 0
            per_chunk = []
            for ci, n in enumerate(chunks[i]):
                gt = pool.tile([128, _slots(n), H], bf16, name=f"g{i + 1}_{ci}")
                ix = its[i][:, col0 : col0 + n // 16]
                nc.gpsimd.dma_gather(gt[:], x[:], ix, n, n, H)
                per_chunk.append((gt, ix, n))
                col0 += n // 16
            gts.append(per_chunk)

        # y3 rows scaled by per-row scalar (f32 table, bf16 data)
        ybs = []
        s0 = 0
        for ci, (gt, ix, n) in enumerate(gts[2]):
            yb = pool.tile([128, _slots(n), H], bf16, name=f"y3b_{ci}")
            for s in range(_slots(n)):
                nc.vector.tensor_scalar(
                    yb[:, s, :], gt[:, s, :], sc[:, s0 + s : s0 + s + 1], None,
                    op0=mul,
                )
            s0 += _slots(n)
            ybs.append(yb)

        for i in range(3):
            for ci, (gt, ix, n) in enumerate(gts[i]):
                src = ybs[ci] if i == 2 else gt
                nc.gpsimd.dma_scatter_add(ys[i][:], src[:], ix, n, n, H)
    nc.finalize()
    return nc


def _wrap16(vals, n):
    """Index list -> int16 [128, n/16] table: j at [j%16, j//16], -1 padded,
    replicated across the 8 gpsimd cores."""
    t = np.full((16, n // 16), -1, dtype=np.int16)
    m = len(vals)
    j = np.arange(m)
    t[j % 16, j // 16] = vals
    return np.tile(t, (8, 1))


def _prep_inputs(bert_local_out, depend, depended, no_connect,
                 depend_weight, depended_weight):
    x = np.asarray(bert_local_out, dtype=np.float32).reshape(B, S, H)
    xb = x.astype(ml_dtypes.bfloat16)
    w1 = np.asarray(depend_weight, dtype=np.float32)
    w2 = np.asarray(depended_weight, dtype=np.float32)

    def row_sets(idx_arr):
        a = np.asarray(idx_arr, dtype=np.int64)
        out = []
        for b in range(B):
            v = a[b]
            v = v[(v >= 0) & (v <= S - 2)]
            out.append(np.unique(v) + 1)
        return out

    D = row_sets(depend)
    P = row_sets(depended)
    N = row_sets(no_connect)

    rows1, rows2, rows3, scal3 = [], [], [], []
    for c in range(N_CORES):
        r1l, r2l, r3l, s3l = [], [], [], []
        for bl in range(BL):
            b = c * BL + bl
            base = bl * S
            r1l.append(base + np.concatenate(([0], D[b])))
            r2l.append(base + np.concatenate(([0], P[b])))
            u = np.union1d(D[b], P[b])
            u = u[~np.isin(u, N[b])]
            sc = np.where(np.isin(u, P[b]), w2[b], w1[b])
            r3l.append(base + np.concatenate(([0], u)))
            s3l.append(np.concatenate(([1.0], sc)).astype(np.float32))
        rows1.append(np.concatenate(r1l))
        rows2.append(np.concatenate(r2l))
        rows3.append(np.concatenate(r3l))
        scal3.append(np.concatenate(s3l))

    def rup16(v):
        return max(16, (v + 15) // 16 * 16)

    n1 = rup16(max(len(r) for r in rows1))
    n2 = rup16(max(len(r) for r in rows2))
    n3 = rup16(max(len(r) for r in rows3))
    ch3 = _chunk_sizes(n3)

    def tables(rows, n):
        """Concat per-chunk wrapped idx tables along columns."""
        parts, at = [], 0
        for cn in _chunk_sizes(n):
            parts.append(_wrap16(rows[at : at + cn], cn))
            at += cn
        return np.concatenate(parts, axis=1)

    in_maps = []
    for c in range(N_CORES):
        sct = np.zeros((128, sum(_slots(cn) for cn in ch3)), dtype=np.float32)
        at = s0 = 0
        for cn in ch3:
            v = scal3[c][at : at + cn]
            j = np.arange(len(v))
            sct[j % 128, s0 + j // 128] = v
            at += cn
            s0 += _slots(cn)
        in_maps.append({
            "x": np.ascontiguousarray(xb[c * BL : (c + 1) * BL]).reshape(ROWS, H),
            "idx1": tables(rows1[c], n1),
            "idx2": tables(rows2[c], n2),
            "idx3": tables(rows3[c], n3),
            "scal": sct,
        })
    return in_maps, (n1, n2, n3)


def kernel(bert_local_out, depend, depended, no_connect,
           depend_weight, depended_weight):
    from concourse.bass_utils import run_bass_kernel_spmd

    in_maps, key = _prep_inputs(bert_local_out, depend, depended, no_connect,
                                depend_weight, depended_weight)
    if key not in _cache:
        _cache.clear()
        _cache[key] = _build(*key)
    nc = _cache[key]

    pdir = os.environ.get("KERNEL_PROFILE_DIR")
    ctx = contextlib.nullcontext()
    if pdir:
        import concourse.bass2jax as b2j
        from trn_agent_boot.trn_boot import _ntff_profile_via_ctypes

        if not getattr(b2j, "_neff_capture_patched", False):
            orig = b2j.rename_neff_tensors_and_patch_header

            def patched(neff_path, mapping):
                data = orig(neff_path, mapping)
                cap = os.environ.get("KERNEL_PROFILE_DIR")
                if cap:
                    os.makedirs(cap, exist_ok=True)
                    with open(os.path.join(cap, "model.neff"), "wb") as f:
                        f.write(data)
                return data

            b2j.rename_neff_tensors_and_patch_header = patched
            b2j._neff_capture_patched = True
        os.makedirs(pdir, exist_ok=True)
        hookf = _ntff_profile_via_ctypes("/opt/axon/libaxon_pjrt.so")
        if hookf is not None:
            dev = None if os.environ.get("KERNEL_PROFILE_ALL") else [0]
            ctx = hookf(pdir, dev)

    with ctx:
        res = run_bass_kernel_spmd(nc, in_maps, list(range(N_CORES)))

    outs = []
    for name in ("y1", "y2", "y3"):
        full = np.empty((B, S, H), dtype=np.float32)
        for c in range(N_CORES):
            full[c * BL : (c + 1) * BL] = (
                np.asarray(res.results[c][name])
                .astype(np.float32)
                .reshape(BL, S, H)
            )
        outs.append(full)
    return tuple(outs)


# revision 14
# speedup vs baseline: 1.8418x; 1.3288x over previous
"""Trainium2 Bass kernel for nn_DLCF_DCA (scatter_memory).

Reference, per sample b (B=128, S=256, H=768, K=64):
  keep_dep[s]  = (s==0) or any_k(depend[b,k] == s-1)
  keep_dpd[s]  = (s==0) or any_k(depended[b,k] == s-1)
  mult[s]      = 1 at s==0; 0 if s-1 in no_connect; else w2 if s-1 in
                 depended, else w1 if s-1 in depend, else 0
  y1 = x * keep_dep;  y2 = x * keep_dpd;  y3 = x * mult

All three outputs are mostly zero rows (~22-31% nonzero).  The runtime
donates pre-zeroed output buffers (bass2jax.run_bass_via_pjrt), so the
kernel only touches the nonzero rows, in bf16 (rel err ~2^-8 << 2e-2):

  per core (16 samples, x shard [4096, 768] bf16):
    dma_gather the nonzero row sets from HBM into SBUF (row j at
    partition j%128, slot j//128), scale y3 rows by a per-row scalar
    table on the vector engine, then dma_scatter_add onto the zeroed
    outputs.  Index tables are int16 [128, N/16] (j at [j%16, j//16],
    replicated across gpsimd cores) padded with trailing -1 which the
    ucode trims per core at runtime; the same table drives both the
    gather and the scatter (y[r] = x[r] * scale).

Q7 descriptor generation (~9 ns/row) is the dominant cost, so the 8
SWDGE instructions are spread over 4 SWDGE queues.  num_idxs per
instruction is capped at 1024, so y3 (~1300 rows) is split in two
position chunks; the chunks scatter into two separate full-size output
tensors (y3 = y3a + y3b on the host) so no write-after-write ordering
serializes them on device.
"""

import contextlib
import os
import sys

import numpy as np

if "/opt/trn_rl_repo" not in sys.path:
    sys.path.insert(0, "/opt/trn_rl_repo")

import ml_dtypes

N_CORES = 8
B, S, H, K = 128, 256, 768, 64
BL = B // N_CORES          # samples per core
ROWS = BL * S              # 4096 rows per core

MAXN = 1024  # dma_gather/scatter_add limit on num_idxs per instruction

_cache = {}


def _chunk_sizes(n):
    """Split n into even 16-aligned chunks of at most MAXN."""
    k = (n + MAXN - 1) // MAXN
    per = ((n + k - 1) // k + 15) // 16 * 16
    out, left = [], n
    for _ in range(k):
        out.append(min(per, max(16, (left + 15) // 16 * 16)))
        left -= out[-1]
    return out


def _slots(n):
    return (n + 127) // 128


def _build(n1, n2, n3):
    """n1/n2/n3: static num_idxs per stream (multiples of 16)."""
    import concourse.bacc as bacc
    import concourse.tile as tile
    from concourse import mybir

    f32 = mybir.dt.float32
    bf16 = mybir.dt.bfloat16
    i16 = mybir.dt.int16
    mul = mybir.AluOpType.mult

    chunks = [_chunk_sizes(n) for n in (n1, n2, n3)]
    cols = [sum(c // 16 for c in ch) for ch in chunks]
    sl3 = sum(_slots(c) for c in chunks[2])

    nc = bacc.Bacc(None, num_swdge_queues=4)
    x = nc.dram_tensor("x", [ROWS, H], bf16, kind="ExternalInput")
    idxs = [nc.dram_tensor(f"idx{i + 1}", [128, cols[i]], i16,
                           kind="ExternalInput") for i in range(3)]
    scal = nc.dram_tensor("scal", [128, sl3], f32, kind="ExternalInput")
    # one output tensor per scatter instruction: no WAW ordering on device;
    # host sums the y3 chunk tensors (disjoint rows, zeros elsewhere).
    outs = {}
    for i in range(3):
        for ci in range(len(chunks[i])):
            outs[(i, ci)] = nc.dram_tensor(
                f"y{i + 1}{'abcd'[ci]}", [ROWS, H], bf16,
                kind="ExternalOutput")

    with tile.TileContext(nc) as tc, contextlib.ExitStack() as ctx:
        pool = ctx.enter_context(tc.tile_pool(name="pool", bufs=1))

        its = []
        for i in range(3):
            it = pool.tile([128, cols[i]], i16, name=f"i{i + 1}")
            nc.sync.dma_start(out=it[:], in_=idxs[i][:])
            its.append(it)
        sc = pool.tile([128, sl3], f32, name="sc")
        nc.sync.dma_start(out=sc[:], in_=scal[:])

        # y3 gathers first: their scatters have the longest dep chain
        order = [(2, ci) for ci in range(len(chunks[2]))] + \
                [(0, ci) for ci in range(len(chunks[0]))] + \
                [(1, ci) for ci in range(len(chunks[1]))]
        colof = {}
        for i in range(3):
            c0 = 0
            for ci, n in enumerate(chunks[i]):
                colof[(i, ci)] = c0
                c0 += n // 16

        gts = {}
        q = 0
        for i, ci in order:
            n = chunks[i][ci]
            gt = pool.tile([128, _slots(n), H], bf16, name=f"g{i + 1}_{ci}")
            ix = its[i][:, colof[(i, ci)] : colof[(i, ci)] + n // 16]
            nc.gpsimd.dma_gather(gt[:], x[:], ix, n, n, H, queue_num=q % 4)
            gts[(i, ci)] = (gt, ix)
            q += 1

        # y3 rows scaled by per-row scalar (f32 table, bf16 data)
        ybs = {}
        s0 = 0
        for ci, n in enumerate(chunks[2]):
            gt, _ = gts[(2, ci)]
            yb = pool.tile([128, _slots(n), H], bf16, name=f"y3b_{ci}")
            for s in range(_slots(n)):
                nc.vector.tensor_scalar(
                    yb[:, s, :], gt[:, s, :], sc[:, s0 + s : s0 + s + 1],
                    None, op0=mul,
                )
            s0 += _slots(n)
            ybs[ci] = yb

        q = 0
        for i, ci in order:
            n = chunks[i][ci]
            gt, ix = gts[(i, ci)]
            src = ybs[ci] if i == 2 else gt
            nc.gpsimd.dma_scatter_add(outs[(i, ci)][:], src[:], ix, n, n, H,
                                      queue_num=q % 4)
            q += 1
    nc.finalize()
    return nc


def _wrap16(vals, n):
    """Index list -> int16 [128, n/16] table: j at [j%16, j//16], -1 padded,
    replicated across the 8 gpsimd cores."""
    t = np.full((16, n // 16), -1, dtype=np.int16)
    m = len(vals)
    j = np.arange(m)
    t[j % 16, j // 16] = vals
    return np.tile(t, (8, 1))


def _prep_inputs(bert_local_out, depend, depended, no_connect,
                 depend_weight, depended_weight):
    x = np.asarray(bert_local_out, dtype=np.float32).reshape(B, S, H)
    xb = x.astype(ml_dtypes.bfloat16)
    w1 = np.asarray(depend_weight, dtype=np.float32)
    w2 = np.asarray(depended_weight, dtype=np.float32)

    def row_sets(idx_arr):
        a = np.asarray(idx_arr, dtype=np.int64)
        out = []
        for b in range(B):
            v = a[b]
            v = v[(v >= 0) & (v <= S - 2)]
            out.append(np.unique(v) + 1)
        return out

    D = row_sets(depend)
    P = row_sets(depended)
    N = row_sets(no_connect)

    rows = [[None] * 3 for _ in range(N_CORES)]
    scal3 = [None] * N_CORES
    for c in range(N_CORES):
        r1l, r2l, r3l, s3l = [], [], [], []
        for bl in range(BL):
            b = c * BL + bl
            base = bl * S
            r1l.append(base + np.concatenate(([0], D[b])))
            r2l.append(base + np.concatenate(([0], P[b])))
            u = np.union1d(D[b], P[b])
            u = u[~np.isin(u, N[b])]
            sc = np.where(np.isin(u, P[b]), w2[b], w1[b])
            r3l.append(base + np.concatenate(([0], u)))
            s3l.append(np.concatenate(([1.0], sc)).astype(np.float32))
        rows[c][0] = np.concatenate(r1l)
        rows[c][1] = np.concatenate(r2l)
        rows[c][2] = np.concatenate(r3l)
        scal3[c] = np.concatenate(s3l)

    def rup16(v):
        return max(16, (v + 15) // 16 * 16)

    n1 = rup16(max(len(r[0]) for r in rows))
    n2 = rup16(max(len(r[1]) for r in rows))
    n3 = rup16(max(len(r[2]) for r in rows))

    def tables(rowlist, n):
        parts, at = [], 0
        for cn in _chunk_sizes(n):
            parts.append(_wrap16(rowlist[at : at + cn], cn))
            at += cn
        return np.concatenate(parts, axis=1)

    in_maps = []
    for c in range(N_CORES):
        m = {"x": np.ascontiguousarray(xb[c * BL : (c + 1) * BL]).reshape(ROWS, H)}
        for i in range(3):
            m[f"idx{i + 1}"] = tables(rows[c][i], (n1, n2, n3)[i])
        ch3 = _chunk_sizes(n3)
        sct = np.zeros((128, sum(_slots(cn) for cn in ch3)), dtype=np.float32)
        at = s0 = 0
        for cn in ch3:
            v = scal3[c][at : at + cn]
            j = np.arange(len(v))
            sct[j % 128, s0 + j // 128] = v
            at += cn
            s0 += _slots(cn)
        m["scal"] = sct
        in_maps.append(m)
    return in_maps, (n1, n2, n3)


def kernel(bert_local_out, depend, depended, no_connect,
           depend_weight, depended_weight):
    from concourse.bass_utils import run_bass_kernel_spmd

    in_maps, key = _prep_inputs(bert_local_out, depend, depended, no_connect,
                                depend_weight, depended_weight)
    if key not in _cache:
        _cache.clear()
        _cache[key] = _build(*key)
    nc = _cache[key]

    pdir = os.environ.get("KERNEL_PROFILE_DIR")
    ctx = contextlib.nullcontext()
    if pdir:
        import concourse.bass2jax as b2j
        from trn_agent_boot.trn_boot import _ntff_profile_via_ctypes

        if not getattr(b2j, "_neff_capture_patched", False):
            orig = b2j.rename_neff_tensors_and_patch_header

            def patched(neff_path, mapping):
                data = orig(neff_path, mapping)
                cap = os.environ.get("KERNEL_PROFILE_DIR")
                if cap:
                    os.makedirs(cap, exist_ok=True)
                    with open(os.path.join(cap, "model.neff"), "wb") as f:
                        f.write(data)
                return data

            b2j.rename_neff_tensors_and_patch_header = patched
            b2j._neff_capture_patched = True
        os.makedirs(pdir, exist_ok=True)
        hookf = _ntff_profile_via_ctypes("/opt/axon/libaxon_pjrt.so")
        if hookf is not None:
            dev = None if os.environ.get("KERNEL_PROFILE_ALL") else [0]
            ctx = hookf(pdir, dev)

    with ctx:
        res = run_bass_kernel_spmd(nc, in_maps, list(range(N_CORES)))

    nchunks = [len(_chunk_sizes(n)) for n in key]
    outs = []
    for i in range(3):
        full = np.empty((B, S, H), dtype=np.float32)
        for c in range(N_CORES):
            acc = np.asarray(res.results[c][f"y{i + 1}a"]).astype(np.float32)
            for ci in range(1, nchunks[i]):
                acc += np.asarray(
                    res.results[c][f"y{i + 1}{'abcd'[ci]}"]).astype(np.float32)
            full[c * BL : (c + 1) * BL] = acc.reshape(BL, S, H)
        outs.append(full)
    return tuple(outs)


# revision 16
# speedup vs baseline: 1.9218x; 1.0435x over previous
"""Trainium2 Bass kernel for nn_DLCF_DCA (scatter_memory).

Reference, per sample b (B=128, S=256, H=768, K=64):
  keep_dep[s]  = (s==0) or any_k(depend[b,k] == s-1)
  keep_dpd[s]  = (s==0) or any_k(depended[b,k] == s-1)
  mult[s]      = 1 at s==0; 0 if s-1 in no_connect; else w2 if s-1 in
                 depended, else w1 if s-1 in depend, else 0
  y1 = x * keep_dep;  y2 = x * keep_dpd;  y3 = x * mult

All three outputs are mostly zero rows (~22-31% nonzero).  The runtime
donates pre-zeroed output buffers (bass2jax.run_bass_via_pjrt), so the
kernel only touches the nonzero rows, in bf16 (rel err ~2^-8 << 2e-2):

  per core (16 samples, x shard [4096, 768] bf16):
    dma_gather the nonzero row sets from HBM into SBUF (row j at
    partition j%128, slot j//128), scale y3 rows by a per-row scalar
    table on the vector engine, then dma_scatter_add onto the zeroed
    outputs.  Index tables are int16 [128, N/16] (j at [j%16, j//16],
    replicated across gpsimd cores) padded with trailing -1 which the
    ucode trims per core at runtime; the same table drives both the
    gather and the scatter (y[r] = x[r] * scale).

Q7 descriptor generation (~9 ns/row) is the dominant cost, so the 8
SWDGE instructions are spread over 4 SWDGE queues.  num_idxs per
instruction is capped at 1024, so y3 (~1300 rows) is split in two
position chunks; the chunks scatter into two separate full-size output
tensors (y3 = y3a + y3b on the host) so no write-after-write ordering
serializes them on device.
"""

import contextlib
import os
import sys

import numpy as np

if "/opt/trn_rl_repo" not in sys.path:
    sys.path.insert(0, "/opt/trn_rl_repo")

import ml_dtypes

N_CORES = 8
B, S, H, K = 128, 256, 768, 64
BL = B // N_CORES          # samples per core
ROWS = BL * S              # 4096 rows per core

MAXN = 1024  # dma_gather/scatter_add limit on num_idxs per instruction

_cache = {}


def _chunk_sizes(n):
    """Split n into even 16-aligned chunks of at most MAXN."""
    k = (n + MAXN - 1) // MAXN
    per = ((n + k - 1) // k + 15) // 16 * 16
    out, left = [], n
    for _ in range(k):
        out.append(min(per, max(16, (left + 15) // 16 * 16)))
        left -= out[-1]
    return out


def _slots(n):
    return (n + 127) // 128


def _build(n1, n2, n3):
    """n1/n2/n3: static num_idxs per stream (multiples of 16)."""
    import concourse.bacc as bacc
    import concourse.tile as tile
    from concourse import mybir

    f32 = mybir.dt.float32
    bf16 = mybir.dt.bfloat16
    i16 = mybir.dt.int16
    mul = mybir.AluOpType.mult

    chunks = [_chunk_sizes(n) for n in (n1, n2, n3)]
    cols = [sum(c // 16 for c in ch) for ch in chunks]
    sl3 = sum(_slots(c) for c in chunks[2])

    nc = bacc.Bacc(None, num_swdge_queues=4)
    x = nc.dram_tensor("x", [ROWS, H], bf16, kind="ExternalInput")
    idxs = [nc.dram_tensor(f"idx{i + 1}", [128, cols[i]], i16,
                           kind="ExternalInput") for i in range(3)]
    scal = nc.dram_tensor("scal", [128, sl3], f32, kind="ExternalInput")
    # one output tensor per scatter instruction: no WAW ordering on device;
    # host sums the y3 chunk tensors (disjoint rows, zeros elsewhere).
    outs = {}
    for i in range(3):
        for ci in range(len(chunks[i])):
            outs[(i, ci)] = nc.dram_tensor(
                f"y{i + 1}{'abcd'[ci]}", [ROWS, H], bf16,
                kind="ExternalOutput")

    with tile.TileContext(nc) as tc, contextlib.ExitStack() as ctx:
        pool = ctx.enter_context(tc.tile_pool(name="pool", bufs=1))

        its = []
        for i in range(3):
            it = pool.tile([128, cols[i]], i16, name=f"i{i + 1}")
            nc.sync.dma_start(out=it[:], in_=idxs[i][:])
            its.append(it)
        sc = pool.tile([128, sl3], f32, name="sc")
        nc.sync.dma_start(out=sc[:], in_=scal[:])

        # y3 gathers first: their scatters have the longest dep chain
        order = [(2, ci) for ci in range(len(chunks[2]))] + \
                [(0, ci) for ci in range(len(chunks[0]))] + \
                [(1, ci) for ci in range(len(chunks[1]))]
        colof = {}
        for i in range(3):
            c0 = 0
            for ci, n in enumerate(chunks[i]):
                colof[(i, ci)] = c0
                c0 += n // 16

        gts = {}
        q = 0
        for i, ci in order:
            n = chunks[i][ci]
            gt = pool.tile([128, _slots(n), H], bf16, name=f"g{i + 1}_{ci}")
            ix = its[i][:, colof[(i, ci)] : colof[(i, ci)] + n // 16]
            nc.gpsimd.dma_gather(gt[:], x[:], ix, n, n, H, queue_num=q % 4)
            gts[(i, ci)] = (gt, ix)
            q += 1

        # y3 rows scaled by per-row scalar (f32 table, bf16 data)
        ybs = {}
        s0 = 0
        for ci, n in enumerate(chunks[2]):
            gt, _ = gts[(2, ci)]
            yb = pool.tile([128, _slots(n), H], bf16, name=f"y3b_{ci}")
            for s in range(_slots(n)):
                nc.vector.tensor_scalar(
                    yb[:, s, :], gt[:, s, :], sc[:, s0 + s : s0 + s + 1],
                    None, op0=mul,
                )
            s0 += _slots(n)
            ybs[ci] = yb

        q = 0
        for i, ci in order:
            n = chunks[i][ci]
            gt, ix = gts[(i, ci)]
            src = ybs[ci] if i == 2 else gt
            nc.gpsimd.dma_scatter_add(outs[(i, ci)][:], src[:], ix, n, n, H,
                                      queue_num=q % 4)
            q += 1
    nc.finalize()
    return nc


def _wrap16(vals, n):
    """Index list -> int16 [128, n/16] table: j at [j%16, j//16], -1 padded,
    replicated across the 8 gpsimd cores."""
    t = np.full((16, n // 16), -1, dtype=np.int16)
    m = len(vals)
    j = np.arange(m)
    t[j % 16, j // 16] = vals
    return np.tile(t, (8, 1))


def _prep_inputs(bert_local_out, depend, depended, no_connect,
                 depend_weight, depended_weight):
    x = np.asarray(bert_local_out, dtype=np.float32).reshape(B, S, H)
    xb = x.astype(ml_dtypes.bfloat16)
    w1 = np.asarray(depend_weight, dtype=np.float32)
    w2 = np.asarray(depended_weight, dtype=np.float32)

    def row_sets(idx_arr):
        a = np.asarray(idx_arr, dtype=np.int64)
        out = []
        for b in range(B):
            v = a[b]
            v = v[(v >= 0) & (v <= S - 2)]
            out.append(np.unique(v) + 1)
        return out

    D = row_sets(depend)
    P = row_sets(depended)
    N = row_sets(no_connect)

    rows = [[None] * 3 for _ in range(N_CORES)]
    scal3 = [None] * N_CORES
    for c in range(N_CORES):
        r1l, r2l, r3l, s3l = [], [], [], []
        for bl in range(BL):
            b = c * BL + bl
            base = bl * S
            r1l.append(base + np.concatenate(([0], D[b])))
            r2l.append(base + np.concatenate(([0], P[b])))
            u = np.union1d(D[b], P[b])
            u = u[~np.isin(u, N[b])]
            sc = np.where(np.isin(u, P[b]), w2[b], w1[b])
            r3l.append(base + np.concatenate(([0], u)))
            s3l.append(np.concatenate(([1.0], sc)).astype(np.float32))
        rows[c][0] = np.concatenate(r1l)
        rows[c][1] = np.concatenate(r2l)
        rows[c][2] = np.concatenate(r3l)
        scal3[c] = np.concatenate(s3l)

    def rup16(v):
        return max(16, (v + 15) // 16 * 16)

    n1 = rup16(max(len(r[0]) for r in rows))
    n2 = rup16(max(len(r[1]) for r in rows))
    n3 = rup16(max(len(r[2]) for r in rows))

    def tables(rowlist, n):
        parts, at = [], 0
        for cn in _chunk_sizes(n):
            parts.append(_wrap16(rowlist[at : at + cn], cn))
            at += cn
        return np.concatenate(parts, axis=1)

    in_maps = []
    for c in range(N_CORES):
        m = {"x": np.ascontiguousarray(xb[c * BL : (c + 1) * BL]).reshape(ROWS, H)}
        for i in range(3):
            m[f"idx{i + 1}"] = tables(rows[c][i], (n1, n2, n3)[i])
        ch3 = _chunk_sizes(n3)
        sct = np.zeros((128, sum(_slots(cn) for cn in ch3)), dtype=np.float32)
        at = s0 = 0
        for cn in ch3:
            v = scal3[c][at : at + cn]
            j = np.arange(len(v))
            sct[j % 128, s0 + j // 128] = v
            at += cn
            s0 += _slots(cn)
        m["scal"] = sct
        in_maps.append(m)
    return in_maps, (n1, n2, n3)


def kernel(bert_local_out, depend, depended, no_connect,
           depend_weight, depended_weight):
    from concourse.bass_utils import run_bass_kernel_spmd

    in_maps, key = _prep_inputs(bert_local_out, depend, depended, no_connect,
                                depend_weight, depended_weight)
    if key not in _cache:
        _cache.clear()
        _cache[key] = _build(*key)
    nc = _cache[key]

    pdir = os.environ.get("KERNEL_PROFILE_DIR")
    ctx = contextlib.nullcontext()
    if pdir:
        import concourse.bass2jax as b2j
        from trn_agent_boot.trn_boot import _ntff_profile_via_ctypes

        if not getattr(b2j, "_neff_capture_patched", False):
            orig = b2j.rename_neff_tensors_and_patch_header

            def patched(neff_path, mapping):
                data = orig(neff_path, mapping)
                cap = os.environ.get("KERNEL_PROFILE_DIR")
                if cap:
                    os.makedirs(cap, exist_ok=True)
                    with open(os.path.join(cap, "model.neff"), "wb") as f:
                        f.write(data)
                return data

            b2j.rename_neff_tensors_and_patch_header = patched
            b2j._neff_capture_patched = True
        os.makedirs(pdir, exist_ok=True)
        hookf = _ntff_profile_via_ctypes("/opt/axon/libaxon_pjrt.so")
        if hookf is not None:
            dev = None if os.environ.get("KERNEL_PROFILE_ALL") else [0]
            ctx = hookf(pdir, dev)

    with ctx:
        res = run_bass_kernel_spmd(nc, in_maps, list(range(N_CORES)))

    nchunks = [len(_chunk_sizes(n)) for n in key]
    outs = []
    for i in range(3):
        full = np.empty((B, S, H), dtype=np.float32)
        for c in range(N_CORES):
            acc = np.asarray(res.results[c][f"y{i + 1}a"]).astype(np.float32)
            for ci in range(1, nchunks[i]):
                acc += np.asarray(
                    res.results[c][f"y{i + 1}{'abcd'[ci]}"]).astype(np.float32)
            full[c * BL : (c + 1) * BL] = acc.reshape(BL, S, H)
        outs.append(full)
    return tuple(outs)
